# revision 4
# baseline (speedup 1.0000x reference)
"""FlowerAttention Trainium2 kernel (8 NeuronCores, tensor-parallel).

Problem: y = proj(attn(rmsnorm+rope(qkv(x)))) with
  x [4, 2048, 2048], w_qkv [6144, 2048], w_proj [2048, 2048],
  per-head RMSNorm on q/k (head_dim=128, eps 1e-6), half-split RoPE
  (theta=32), dense softmax attention (no mask), output projection.

Sharding: core c -> (batch b = c//2, head-group g = c%2 of 8 heads).
Each core computes the full pipeline for its (b, g); the output
projection contracts only the local 1024 head-dims, so the host sums
the two partial outputs per batch.

Device-side layout strategy (all matmuls in float32r = full-rate fp32):
 - Host pre-transposes x -> xT [d, s] and weight slices -> wT [d, e].
 - Phase V/Q/K: QKV matmuls produce V in natural [s, e] layout and
   Q^T/K^T in [head_dim, s] layout directly (no transposes on device).
   RMSNorm is folded as a per-(position,head) scalar applied after RoPE
   (RoPE is a rotation, so it commutes with the scalar); the norm
   weights are folded into host-precomputed cos/sin tables. The
   sum-of-squares over head_dim (the partition axis) is done with a
   ones-matmul on the PE, which also replicates it across partitions.
   The RoPE half-swap is a single permutation-matmul on the PE.
 - Phase attention (per head): E^T tile [k_seq, q_seq-block] =
   exp(scale * K^T.T @ Q^T) via PE + ScalarE; row-sums r accumulate on
   VectorE + a ones-matmul; O^T [head_dim, q_seq] accumulates via
   lhsT=V_kt, rhs=E^T_kt, then is normalized by 1/r.
 - Phase proj: out[s, e] partial = sum_h O^T_h.T @ w_projT, straight
   from the O^T layout.
"""

import os
import sys

for _p in (
    "/root/.axon_site",
    "/root/.axon_site/_ro/trn_rl_repo",
    "/root/.axon_site/_ro/pypackages",
    "/opt/trn_rl_repo",
):
    if os.path.isdir(_p) and _p not in sys.path:
        sys.path.append(_p)

import numpy as np

import concourse.bass as bass  # noqa: F401  (bass types used via tile/bacc)
import concourse.tile as tile
from concourse import bacc, mybir
from concourse.bass_utils import run_bass_kernel_spmd

F32 = mybir.dt.float32
F32R = mybir.dt.float32r
AF = mybir.ActivationFunctionType

DIM = 2048
N_HEADS = 16
HEAD_DIM = 128
SEQ = 2048
BSZ = 4
THETA = 32.0
EPS = 1e-6
N_CORES = 8
HPC = 8  # heads per core
LOCAL_E = HPC * HEAD_DIM  # 1024
SCALE = HEAD_DIM ** -0.5
P = 128
KT = DIM // P  # 16 contraction subtiles over model dim
SB = 512  # free-dim block
NSB = SEQ // SB  # 4 seq blocks
NST = SEQ // P  # 16 seq tiles


def _build_program():
    nc = bacc.Bacc()

    xt = nc.dram_tensor("xt", [DIM, SEQ], F32R, kind="ExternalInput")
    wq = nc.dram_tensor("wq", [DIM, LOCAL_E], F32R, kind="ExternalInput")
    wk = nc.dram_tensor("wk", [DIM, LOCAL_E], F32R, kind="ExternalInput")
    wv = nc.dram_tensor("wv", [DIM, LOCAL_E], F32R, kind="ExternalInput")
    wp = nc.dram_tensor("wp", [LOCAL_E, DIM], F32R, kind="ExternalInput")
    cq = nc.dram_tensor("cq", [P, SEQ], F32, kind="ExternalInput")
    sq = nc.dram_tensor("sq", [P, SEQ], F32, kind="ExternalInput")
    ck = nc.dram_tensor("ck", [P, SEQ], F32, kind="ExternalInput")
    sk = nc.dram_tensor("sk", [P, SEQ], F32, kind="ExternalInput")
    swap = nc.dram_tensor("swap", [P, P], F32R, kind="ExternalInput")
    out = nc.dram_tensor("out", [SEQ, DIM], F32, kind="ExternalOutput")

    xt_re = xt[:].rearrange("(kt p) s -> p kt s", p=P)
    wv_re = wv[:].rearrange("(kt p) e -> p kt e", p=P)
    wp_re = wp[:].rearrange("(h p) e -> p h e", p=P)

    with tile.TileContext(nc) as tc:
        with (
            tc.tile_pool(name="dram", bufs=1, space="DRAM") as dram,
            tc.tile_pool(name="const", bufs=1) as const,
        ):
            qT_s = dram.tile([HPC, P, SEQ], F32R)
            kT_s = dram.tile([HPC, P, SEQ], F32R)
            v_s = dram.tile([SEQ, LOCAL_E], F32R)
            v_s_re = v_s[:].rearrange("(kt p) e -> p kt e", p=P)

            eps_t = const.tile([P, 1], F32)
            nc.vector.memset(eps_t[:], EPS)
            ones_f = const.tile([P, P], F32)
            nc.vector.memset(ones_f[:], 1.0)
            ones_r = const.tile([P, P], F32R)
            nc.vector.tensor_copy(ones_r[:], ones_f[:])
            swap_t = const.tile([P, P], F32R)
            nc.sync.dma_start(out=swap_t[:], in_=swap[:])

            # ---------------- Phase V: v = x @ wv^T (natural layout) -------
            with (
                tc.tile_pool(name="wv_pool", bufs=1) as wv_pool,
                tc.tile_pool(name="v_tmp", bufs=3) as v_tmp,
                tc.tile_pool(name="v_psum", bufs=2, space="PSUM") as v_psum,
            ):
                wv_sb = wv_pool.tile([P, KT, LOCAL_E], F32R)
                nc.sync.dma_start(out=wv_sb[:], in_=wv_re)
                for st in range(NST):
                    xt_col = v_tmp.tile([P, KT, P], F32R, tag="xtc")
                    nc.sync.dma_start(
                        out=xt_col[:], in_=xt_re[:, :, st * P : (st + 1) * P]
                    )
                    for vb in range(LOCAL_E // SB):
                        ps_v = v_psum.tile([P, SB], F32)
                        for kt in range(KT):
                            nc.tensor.matmul(
                                ps_v[:],
                                xt_col[:, kt, :],
                                wv_sb[:, kt, vb * SB : (vb + 1) * SB],
                                start=(kt == 0),
                                stop=(kt == KT - 1),
                            )
                        vsb = v_tmp.tile([P, SB], F32R, tag="vsb")
                        nc.vector.tensor_copy(vsb[:], ps_v[:])
                        nc.sync.dma_start(
                            out=v_s[st * P : (st + 1) * P, vb * SB : (vb + 1) * SB],
                            in_=vsb[:],
                        )

            # ---------------- Phases Q / K: transposed + RMS + RoPE --------
            def qk_pass(w_dram, c_dram, s_dram, dst):
                with (
                    tc.tile_pool(name="w_pool", bufs=1) as w_pool,
                    tc.tile_pool(name="cs_pool", bufs=1) as cs_pool,
                    tc.tile_pool(name="x_pool", bufs=2) as x_pool,
                    tc.tile_pool(name="t_pool", bufs=3) as t_pool,
                    tc.tile_pool(name="ps_a", bufs=2, space="PSUM") as ps_a,
                    tc.tile_pool(name="ps_b", bufs=2, space="PSUM") as ps_b,
                    tc.tile_pool(name="ps_c", bufs=2, space="PSUM") as ps_c,
                ):
                    w_re = w_dram[:].rearrange("(kt p) e -> p kt e", p=P)
                    w_sb = []
                    for h in range(HPC):
                        wt = w_pool.tile([P, KT, P], F32R, tag=f"w{h}")
                        nc.sync.dma_start(
                            out=wt[:], in_=w_re[:, :, h * P : (h + 1) * P]
                        )
                        w_sb.append(wt)
                    ctab = cs_pool.tile([P, SEQ], F32)
                    nc.sync.dma_start(out=ctab[:], in_=c_dram[:])
                    stab = cs_pool.tile([P, SEQ], F32)
                    nc.sync.dma_start(out=stab[:], in_=s_dram[:])

                    for sb in range(NSB):
                        ss = slice(sb * SB, (sb + 1) * SB)
                        xt_sb = x_pool.tile([P, KT, SB], F32R, tag="xtsb")
                        nc.sync.dma_start(out=xt_sb[:], in_=xt_re[:, :, ss])
                        for h in range(HPC):
                            ps_q = ps_a.tile([P, SB], F32)
                            for kt in range(KT):
                                nc.tensor.matmul(
                                    ps_q[:],
                                    w_sb[h][:, kt, :],
                                    xt_sb[:, kt, :],
                                    start=(kt == 0),
                                    stop=(kt == KT - 1),
                                )
                            qt = t_pool.tile([P, SB], F32R, tag="qt")
                            nc.vector.tensor_copy(qt[:], ps_q[:])
                            sqt = t_pool.tile([P, SB], F32R, tag="sqt")
                            nc.scalar.activation(
                                out=sqt[:], in_=ps_q[:], func=AF.Square
                            )
                            ps_ms = ps_b.tile([P, SB], F32)
                            nc.tensor.matmul(
                                ps_ms[:], ones_r[:], sqt[:], start=True, stop=True
                            )
                            rms = t_pool.tile([P, SB], F32, tag="rms")
                            nc.scalar.activation(
                                out=rms[:],
                                in_=ps_ms[:],
                                func=AF.Sqrt,
                                scale=1.0 / HEAD_DIM,
                                bias=eps_t[:],
                            )
                            inv = t_pool.tile([P, SB], F32, tag="inv")
                            nc.vector.reciprocal(inv[:], rms[:])
                            ps_rot = ps_c.tile([P, SB], F32)
                            nc.tensor.matmul(
                                ps_rot[:], swap_t[:], qt[:], start=True, stop=True
                            )
                            t1 = t_pool.tile([P, SB], F32, tag="t1")
                            nc.vector.tensor_mul(
                                t1[:], qt[:].bitcast(F32), ctab[:, ss]
                            )
                            t2 = t_pool.tile([P, SB], F32, tag="t2")
                            nc.vector.tensor_mul(t2[:], ps_rot[:], stab[:, ss])
                            nc.vector.tensor_add(t1[:], t1[:], t2[:])
                            qr = t_pool.tile([P, SB], F32R, tag="qr")
                            nc.vector.tensor_mul(qr[:], t1[:], inv[:])
                            nc.sync.dma_start(out=dst[h, :, ss], in_=qr[:])

            qk_pass(wq, cq, sq, qT_s)
            qk_pass(wk, ck, sk, kT_s)

            # ---------------- Phase attention (per head) -------------------
            with tc.tile_pool(name="oT_pool", bufs=1) as oT_pool:
                oT = oT_pool.tile([P, HPC, SEQ], F32R)
                with (
                    tc.tile_pool(name="head_pool", bufs=2) as head_pool,
                    tc.tile_pool(name="e_pool", bufs=2) as e_pool,
                    tc.tile_pool(name="a_tmp", bufs=3) as a_tmp,
                    tc.tile_pool(name="ps_e", bufs=3, space="PSUM") as ps_e_pool,
                    tc.tile_pool(name="ps_r", bufs=2, space="PSUM") as ps_r_pool,
                    tc.tile_pool(name="ps_o", bufs=2, space="PSUM") as ps_o_pool,
                ):
                    for h in range(HPC):
                        qh = head_pool.tile([P, SEQ], F32R, tag="qh")
                        nc.sync.dma_start(out=qh[:], in_=qT_s[h])
                        kh = head_pool.tile([P, SEQ], F32R, tag="kh")
                        nc.sync.dma_start(out=kh[:], in_=kT_s[h])
                        vh = head_pool.tile([P, KT, P], F32R, tag="vh")
                        nc.sync.dma_start(
                            out=vh[:], in_=v_s_re[:, :, h * P : (h + 1) * P]
                        )
                        for qb in range(NSB):
                            qs = slice(qb * SB, (qb + 1) * SB)
                            e_all = e_pool.tile([P, KT, SB], F32R, tag="eall")
                            racc = a_tmp.tile([P, SB], F32, tag="racc")
                            for kt in range(KT):
                                ps_e = ps_e_pool.tile([P, SB], F32)
                                nc.tensor.matmul(
                                    ps_e[:],
                                    kh[:, kt * P : (kt + 1) * P],
                                    qh[:, qs],
                                    start=True,
                                    stop=True,
                                )
                                nc.scalar.activation(
                                    out=e_all[:, kt, :],
                                    in_=ps_e[:],
                                    func=AF.Exp,
                                    scale=SCALE,
                                )
                                if kt == 0:
                                    nc.vector.tensor_copy(racc[:], e_all[:, 0, :])
                                else:
                                    nc.vector.tensor_add(
                                        racc[:],
                                        racc[:],
                                        e_all[:, kt, :].bitcast(F32),
                                    )
                            ps_rr = ps_r_pool.tile([P, SB], F32)
                            nc.tensor.matmul(
                                ps_rr[:], ones_f[:], racc[:], start=True, stop=True
                            )
                            invr = a_tmp.tile([P, SB], F32, tag="invr")
                            nc.vector.reciprocal(invr[:], ps_rr[:])
                            ps_o = ps_o_pool.tile([P, SB], F32)
                            for kt in range(KT):
                                nc.tensor.matmul(
                                    ps_o[:],
                                    vh[:, kt, :],
                                    e_all[:, kt, :],
                                    start=(kt == 0),
                                    stop=(kt == KT - 1),
                                )
                            nc.vector.tensor_mul(oT[:, h, qs], ps_o[:], invr[:])

                # ---------------- Phase proj -------------------------------
                with (
                    tc.tile_pool(name="wp_pool", bufs=1) as wp_pool,
                    tc.tile_pool(name="p_tmp", bufs=3) as p_tmp,
                    tc.tile_pool(name="ps_p", bufs=2, space="PSUM") as ps_p_pool,
                ):
                    wp_sb = wp_pool.tile([P, HPC, DIM], F32R)
                    nc.sync.dma_start(out=wp_sb[:], in_=wp_re)
                    for st in range(NST):
                        for eb in range(NSB):
                            es = slice(eb * SB, (eb + 1) * SB)
                            ps_p = ps_p_pool.tile([P, SB], F32)
                            for h in range(HPC):
                                nc.tensor.matmul(
                                    ps_p[:],
                                    oT[:, h, st * P : (st + 1) * P],
                                    wp_sb[:, h, es],
                                    start=(h == 0),
                                    stop=(h == HPC - 1),
                                )
                            ob = p_tmp.tile([P, SB], F32, tag="ob")
                            nc.vector.tensor_copy(ob[:], ps_p[:])
                            nc.sync.dma_start(
                                out=out[st * P : (st + 1) * P, es], in_=ob[:]
                            )

    nc.finalize()
    return nc


_PROGRAM = None


def _get_program():
    global _PROGRAM
    if _PROGRAM is None:
        _PROGRAM = _build_program()
    return _PROGRAM


def _rope_tables(norm_w):
    """C/S tables [128, SEQ] for transposed-layout RoPE with the per-head
    norm weight folded in.  out = q*C + rot(q)*S with rot(q)[p] =
    q[(p+64) % 128]."""
    half = HEAD_DIM // 2  # 64
    freqs = THETA ** (-np.arange(0, HEAD_DIM, 2, dtype=np.float32) / HEAD_DIM)
    pos = np.arange(SEQ, dtype=np.float32)
    ang = pos[:, None] * freqs[None, :]  # [SEQ, 64]
    cos = np.cos(ang).astype(np.float32)  # [SEQ, 64]
    sin = np.sin(ang).astype(np.float32)
    w = np.asarray(norm_w, dtype=np.float32)
    C = np.empty((P, SEQ), dtype=np.float32)
    S = np.empty((P, SEQ), dtype=np.float32)
    for p in range(P):
        C[p] = cos[:, p % half] * w[p]
    for p in range(half):
        S[p] = -sin[:, p] * w[p + half]
    for p in range(half, P):
        S[p] = sin[:, p - half] * w[p - half]
    return C, S


def kernel(x, w_qkv, w_proj, q_norm_w, k_norm_w):
    x = np.asarray(x, dtype=np.float32)
    w_qkv = np.asarray(w_qkv, dtype=np.float32)
    w_proj = np.asarray(w_proj, dtype=np.float32)

    nc = _get_program()

    cq_t, sq_t = _rope_tables(q_norm_w)
    ck_t, sk_t = _rope_tables(k_norm_w)
    swap_m = np.zeros((P, P), dtype=np.float32)
    half = HEAD_DIM // 2
    for p in range(P):
        swap_m[(p + half) % P, p] = 1.0

    xts = [np.ascontiguousarray(x[b].T) for b in range(BSZ)]
    wqs, wks, wvs, wps = [], [], [], []
    for g in range(2):
        rows = slice(g * LOCAL_E, (g + 1) * LOCAL_E)
        wqs.append(np.ascontiguousarray(w_qkv[rows].T))
        wks.append(np.ascontiguousarray(w_qkv[DIM + g * LOCAL_E : DIM + (g + 1) * LOCAL_E].T))
        wvs.append(np.ascontiguousarray(w_qkv[2 * DIM + g * LOCAL_E : 2 * DIM + (g + 1) * LOCAL_E].T))
        wps.append(np.ascontiguousarray(w_proj[:, rows].T))

    in_maps = []
    for c in range(N_CORES):
        b, g = c // 2, c % 2
        in_maps.append(
            {
                "xt": xts[b],
                "wq": wqs[g],
                "wk": wks[g],
                "wv": wvs[g],
                "wp": wps[g],
                "cq": cq_t,
                "sq": sq_t,
                "ck": ck_t,
                "sk": sk_t,
                "swap": swap_m,
            }
        )

    res = run_bass_kernel_spmd(nc, in_maps, list(range(N_CORES)))
    out = np.empty((BSZ, SEQ, DIM), dtype=np.float32)
    for b in range(BSZ):
        out[b] = res.results[2 * b]["out"] + res.results[2 * b + 1]["out"]
    return out
